# Initial kernel scaffold
#
"""Causal sliding-window attention (B=2, T=2048, D=1024, H=16, W=512) on 8 trn2 cores.

Sequence-parallel sharding: each core owns 512 consecutive tokens of one batch
and recomputes the 512-token halo k/v locally (no collectives). All compute is
feature-major (tokens on the matmul free dim) in float32r:

  xT -> qT/kT (feature-major), v (token-major, with a ones column per head)
  scoresT[keys, q] = kT_h.T-free matmul -> exp on ACT (bias kills chunk-0 halo)
  band masks: 0/1 multiplies on the two diagonal key-tiles per query window
  attV: v-stationary matmul; the ones column yields softmax sums as a psum row
  normalize: DVE reciprocal + partition-broadcast DMA + psum-evicting multiply
  outT = wo-stationary matmul over attT; host transposes/concats core outputs.
"""
import sys

sys.path.insert(0, "/opt/trn_rl_repo")

import numpy as np

B, T, D = 2, 2048, 1024
H, HD, W = 16, 64, 512
NCORES = 8
CHUNK = 512  # own tokens per core
TOK = 2 * CHUNK  # halo + own
NKD = D // 128  # 8 contraction tiles
SCALE = HD ** -0.5

# query-window [qlo, qhi) per key-tile kb, padded to >=256 cols for fp32r rate
QRANGE = []
for kb in range(8):
    qlo = max(0, 128 * kb - 512)
    qhi = min(512, 128 * kb + 128)
    if qhi - qlo < 256:
        qlo, qhi = (0, 256) if qlo == 0 else (256, 512)
    QRANGE.append((qlo, qhi))

_BUILT = None


def _build():
    import concourse.bass as bass
    import concourse.tile as tile
    from concourse import mybir, bacc

    f32 = mybir.dt.float32
    f32r = mybir.dt.float32r

    nc = bacc.Bacc("TRN2", target_bir_lowering=False, debug=False,
                   num_devices=NCORES)
    xT = nc.dram_tensor("xT", [D, TOK], f32r, kind="ExternalInput")
    wq = nc.dram_tensor("wq", [D, D], f32r, kind="ExternalInput")
    wk = nc.dram_tensor("wk", [D, D], f32r, kind="ExternalInput")
    wv = nc.dram_tensor("wv", [D, D], f32r, kind="ExternalInput")
    wo = nc.dram_tensor("wo", [D, D], f32r, kind="ExternalInput")
    # [:, 0, :] = strict-lower-tri (j>q edge), [:, 1, :] = upper-incl
    # (j<=q+W edge), [:, 2, :] = zeros (kills fully-invalid padded cols)
    mask = nc.dram_tensor("mask", [128, 3, 128], f32, kind="ExternalInput")
    vones = nc.dram_tensor("vones", [128, H], f32r, kind="ExternalInput")
    kbias = nc.dram_tensor("kbias", [128, NKD], f32, kind="ExternalInput")
    outT = nc.dram_tensor("outT", [D, CHUNK], f32, kind="ExternalOutput")
    # per-head softmax-recip row, bounced through DRAM to broadcast across
    # partitions (SBUF DMA sources cannot have a zero partition step)
    rscratch = nc.dram_tensor("rscratch", [H, CHUNK], f32, kind="Internal")

    xT_r = xT.rearrange("(kd p) t -> kd p t", p=128)
    w_r = {n: w.rearrange("(kd p) c -> kd p c", p=128)
           for n, w in (("wq", wq), ("wk", wk), ("wv", wv))}
    # wo reshaped so every head's 64 contraction rows sit at partition base 0
    # (matmul needs lhsT and rhs on the same base; attT halves live at 0..63)
    wo_r = wo.rearrange("(hh d) e -> d hh e", hh=H)

    with tile.TileContext(nc) as tc:
        with tc.tile_pool(name="const", bufs=1) as constp, \
             tc.tile_pool(name="qkv", bufs=1) as qkvp, \
             tc.tile_pool(name="ps_mm", bufs=2, space="PSUM") as ps_mm:

            mask_sb = constp.tile([128, 3, 128], f32)
            nc.sync.dma_start(out=mask_sb, in_=mask[:, :, :])
            kbias_sb = constp.tile([128, NKD], f32)
            nc.sync.dma_start(out=kbias_sb, in_=kbias[:, :])

            # ---- persistent qkv buffers (feature-major q/k, token-major v)
            qT_sb = qkvp.tile([128, NKD, CHUNK], f32r)   # q dims x own tokens
            kT_sb = qkvp.tile([128, NKD, TOK], f32r)     # k dims x keys
            # v: per key-tile and head, 65 stationary columns: cols [0:64] = v,
            # col 64 = ones -> attV psum rows 0..63 = att, row 64 = softmax sum
            v_sb = qkvp.tile([128, NKD, H // 2, 2, 65], f32r)

            with tc.tile_pool(name="wts", bufs=2) as wpool, \
                 tc.tile_pool(name="xp", bufs=1) as xp:
                w_tiles = {}
                for wn in ("wq", "wk", "wv"):
                    w_tiles[wn] = [
                        wpool.tile([128, D], f32r, tag=f"w{kd}",
                                   name=f"{wn}_{kd}")
                        for kd in range(NKD)]
                    for kd in range(NKD):
                        nc.sync.dma_start(out=w_tiles[wn][kd], in_=w_r[wn][kd])
                x_tiles = [xp.tile([128, TOK], f32r, tag=f"x{kd}",
                                   name=f"x_{kd}")
                           for kd in range(NKD)]
                for kd in range(NKD):
                    nc.sync.dma_start(out=x_tiles[kd], in_=xT_r[kd])

                # ---- q projection: qT[co] = sum_kd wq[kd,co].T @ xT[kd, own]
                for co in range(NKD):
                    ps = ps_mm.tile([128, CHUNK], f32)
                    for kd in range(NKD):
                        nc.tensor.matmul(
                            ps[:],
                            w_tiles["wq"][kd][:, co * 128:(co + 1) * 128],
                            x_tiles[kd][:, CHUNK:TOK],
                            start=(kd == 0), stop=(kd == NKD - 1))
                    nc.scalar.copy(qT_sb[:, co, :], ps[:])

                # ---- k projection over all TOK keys
                for co in range(NKD):
                    for th in range(2):
                        ps = ps_mm.tile([128, CHUNK], f32)
                        for kd in range(NKD):
                            nc.tensor.matmul(
                                ps[:],
                                w_tiles["wk"][kd][:, co * 128:(co + 1) * 128],
                                x_tiles[kd][:, th * CHUNK:(th + 1) * CHUNK],
                                start=(kd == 0), stop=(kd == NKD - 1))
                        nc.scalar.copy(kT_sb[:, co, th * CHUNK:(th + 1) * CHUNK],
                                       ps[:])

                # ---- v projection, token-major: v[tt] = xT[:,tt].T @ wv
                for tt in range(NKD):
                    for cv in range(2):
                        ps = ps_mm.tile([128, CHUNK], f32)
                        for kd in range(NKD):
                            nc.tensor.matmul(
                                ps[:],
                                x_tiles[kd][:, tt * 128:(tt + 1) * 128],
                                w_tiles["wv"][kd][:, cv * CHUNK:(cv + 1) * CHUNK],
                                start=(kd == 0), stop=(kd == NKD - 1))
                        # scatter 8 heads (cols of 64) into the 65-col slots
                        ps4 = ps[:].rearrange("p (g par d) -> p g par d",
                                              par=2, d=HD)
                        g0 = cv * 4
                        nc.scalar.copy(
                            v_sb[:, tt, g0:g0 + 4, 0, 0:HD], ps4[:, :, 0, :])
                        nc.scalar.copy(
                            v_sb[:, tt, g0:g0 + 4, 1, 0:HD], ps4[:, :, 1, :])
                # ones column of every stationary (memset can't write f32r)
                for tt in range(NKD):
                    nc.sync.dma_start(
                        out=v_sb[:, tt, :, :, HD:HD + 1], in_=vones[:, :])

            # ---- attention + output projection
            with tc.tile_pool(name="attb", bufs=1) as attbp, \
                 tc.tile_pool(name="pt", bufs=3) as ptp, \
                 tc.tile_pool(name="nrm", bufs=2) as nrmp, \
                 tc.tile_pool(name="oev", bufs=1) as oevp, \
                 tc.tile_pool(name="ps_sc", bufs=3, space="PSUM") as ps_sc, \
                 tc.tile_pool(name="ps_at", bufs=3, space="PSUM") as ps_at:

                # att features split by head parity so every head's psum rows
                # (0..63, sums at 64) evict to partitions 0..63
                attT_e = attbp.tile([64, NKD, CHUNK], f32r)
                attT_o = attbp.tile([64, NKD, CHUNK], f32r)
                attT_sb = (attT_e, attT_o)

                # wo with all 16 head-row-groups at partition base 0; loads
                # overlap the attention phase (slot reuses freed x/w space)
                wo_sb = attbp.tile([64, H, D], f32r)
                for g in range(4):
                    nc.sync.dma_start(out=wo_sb[:, 4 * g:4 * (g + 1), :],
                                      in_=wo_r[:, 4 * g:4 * (g + 1), :])

                KB_ORDER = [3, 4, 0, 1, 2, 5, 6, 7]  # first covers q[0:512)
                for h in range(H):
                    hp, po = h // 2, (h % 2) * 64
                    att_ps = ps_at.tile([128, CHUNK], f32)
                    for i, kb in enumerate(KB_ORDER):
                        qlo, qhi = QRANGE[kb]
                        wdt = qhi - qlo
                        sc_ps = ps_sc.tile([128, CHUNK], f32, tag="sc")
                        nc.tensor.matmul(
                            sc_ps[:, 0:wdt],
                            kT_sb[po:po + 64, hp, kb * 128:(kb + 1) * 128],
                            qT_sb[po:po + 64, hp, qlo:qhi],
                            start=True, stop=True)
                        pt = ptp.tile([128, CHUNK], f32r, tag="pt")
                        nc.scalar.activation(
                            pt[:, 0:wdt], sc_ps[:, 0:wdt],
                            mybir.ActivationFunctionType.Exp,
                            bias=kbias_sb[:, kb:kb + 1], scale=SCALE)
                        # band-edge masking on the diagonal 128-col block
                        if kb <= 3:
                            dlo = 128 * kb - qlo
                            nc.vector.tensor_mul(
                                pt[:, dlo:dlo + 128], pt[:, dlo:dlo + 128],
                                mask_sb[:, 0, :])
                            if kb == 0:  # padded cols are fully invalid
                                nc.vector.tensor_mul(
                                    pt[:, 128:256], pt[:, 128:256],
                                    mask_sb[:, 2, :])
                        else:
                            dlo = 128 * (kb - 4) - qlo
                            nc.vector.tensor_mul(
                                pt[:, dlo:dlo + 128], pt[:, dlo:dlo + 128],
                                mask_sb[:, 1, :])
                            if kb == 7:
                                nc.vector.tensor_mul(
                                    pt[:, 0:128], pt[:, 0:128],
                                    mask_sb[:, 2, :])
                        nc.tensor.matmul(
                            att_ps[0:65, qlo:qhi],
                            v_sb[:, kb, hp, h % 2, :],
                            pt[:, 0:wdt],
                            start=(i == 0), stop=(i == len(KB_ORDER) - 1))
                    # normalize: recip of the sums row, broadcast, multiply
                    recip = nrmp.tile([128, CHUNK], f32, tag="recip")
                    nc.vector.reciprocal(recip[64:65, :], att_ps[64:65, :])
                    bc = nrmp.tile([128, CHUNK], f32, tag="bc")
                    nc.sync.dma_start(out=rscratch[h:h + 1, :],
                                      in_=recip[64:65, :])
                    bcast_src = bass.AP(
                        tensor=rscratch, offset=h * CHUNK,
                        ap=[[0, 64], [1, CHUNK]])
                    nc.gpsimd.dma_start(out=bc[0:64, :], in_=bcast_src)
                    nc.vector.tensor_mul(
                        attT_sb[h % 2][0:64, hp, :],
                        att_ps[0:64, :],
                        bc[0:64, :])

                # ---- output projection: 16 K=64 half-matmuls per output tile
                for eo in range(NKD):
                    ps = ps_mm.tile([128, CHUNK], f32)
                    for hh in range(H):
                        nc.tensor.matmul(
                            ps[:],
                            wo_sb[0:64, hh, eo * 128:(eo + 1) * 128],
                            attT_sb[hh % 2][0:64, hh // 2, :],
                            start=(hh == 0), stop=(hh == H - 1))
                    ot = oevp.tile([128, CHUNK], f32, tag="ot")
                    nc.scalar.copy(ot[:], ps[:])
                    nc.sync.dma_start(out=outT[eo * 128:(eo + 1) * 128, :],
                                      in_=ot[:])

    nc.compile()
    return nc


def _host_inputs(x, w_qkv, w_out):
    x = np.ascontiguousarray(np.asarray(x, dtype=np.float32))
    w_qkv = np.ascontiguousarray(np.asarray(w_qkv, dtype=np.float32))
    w_out = np.ascontiguousarray(np.asarray(w_out, dtype=np.float32))

    wq = np.ascontiguousarray(w_qkv[:, 0:D])
    wk = np.ascontiguousarray(w_qkv[:, D:2 * D])
    wv = np.ascontiguousarray(w_qkv[:, 2 * D:3 * D])

    r = np.arange(128)[:, None]
    c = np.arange(128)[None, :]
    mask = np.zeros((128, 3, 128), dtype=np.float32)
    mask[:, 0, :] = (r > c).astype(np.float32)
    mask[:, 1, :] = (r <= c).astype(np.float32)
    vones = np.ones((128, H), dtype=np.float32)

    in_maps = []
    for core in range(NCORES):
        b, qc = divmod(core, 4)
        q0 = qc * CHUNK
        xa = np.zeros((TOK, D), dtype=np.float32)
        lo = max(0, q0 - CHUNK)
        xa[CHUNK - (q0 - lo):] = x[b, lo:q0 + CHUNK]
        kb_bias = np.zeros((128, NKD), dtype=np.float32)
        if qc == 0:
            kb_bias[:, 0:4] = -250.0
        in_maps.append({
            "xT": np.ascontiguousarray(xa.T),
            "wq": wq, "wk": wk, "wv": wv, "wo": w_out,
            "mask": mask, "kbias": kb_bias, "vones": vones,
        })
    return in_maps


def kernel(x, w_qkv, w_out):
    global _BUILT
    if _BUILT is None:
        _BUILT = _build()
    from concourse.bass_utils import run_bass_kernel_spmd

    in_maps = _host_inputs(x, w_qkv, w_out)
    res = run_bass_kernel_spmd(_BUILT, in_maps, core_ids=list(range(NCORES)))
    out = np.empty((B, T, D), dtype=np.float32)
    for core in range(NCORES):
        b, qc = divmod(core, 4)
        out[b, qc * CHUNK:(qc + 1) * CHUNK, :] = res.results[core]["outT"].T
    return out



# revision 4
# speedup vs baseline: 1.1342x; 1.1342x over previous
"""Causal sliding-window attention (B=2, T=2048, D=1024, H=16, W=512) on 8 trn2 cores.

Sequence-parallel: each core owns 512 tokens of one batch, recomputes the
512-token halo k/v locally. Head-paired attention: heads (2hp, 2hp+1) share
kT/qT partition halves; scores for both heads of a pair land in one 2-bank
psum tile so exp processes both at once (half the ACT fixed cost). attV for
the even head accumulates att rows 0..63 + softmax-sum row 64; the odd head
accumulates sum row 63 + att rows 64..127, so the packed attT tile feeds a
K=128 output projection against a host-prearranged pair-major wo.
Projection psum evictions ride DVE; normalization recip broadcasts bounce
through DRAM on the pool queue.
"""
import sys

sys.path.insert(0, "/opt/trn_rl_repo")

import numpy as np

B, T, D = 2, 2048, 1024
H, HD, W = 16, 64, 512
NCORES = 8
CHUNK = 512  # own tokens per core
TOK = 2 * CHUNK  # halo + own
NKD = D // 128  # 8 contraction tiles
NHP = H // 2  # head pairs
SCALE = HD ** -0.5

# query-window [qlo, qhi) per key-tile kb, padded to >=256 cols for fp32r rate
QRANGE = []
for kb in range(8):
    qlo = max(0, 128 * kb - 512)
    qhi = min(512, 128 * kb + 128)
    if qhi - qlo < 256:
        qlo, qhi = (0, 256) if qlo == 0 else (256, 512)
    QRANGE.append((qlo, qhi))

# per kb: one contiguous masked region (col offset rel. qlo, mask slots a:b)
# mask slots: 0 = strict-lower (j>q edge), 1 = zeros, 2 = upper-incl (far edge)
MASKR = {
    0: (0, 0, 2),    # [lower | zeros] over cols 0:256
    1: (128, 0, 1),
    2: (256, 0, 1),
    3: (384, 0, 1),
    4: (0, 2, 3),
    5: (0, 2, 3),
    6: (0, 2, 3),
    7: (0, 1, 3),    # [zeros | upper] over cols 0:256
}

# start with a full-width kb so the psum accumulation init covers all cols
KB_ORDER = [4, 5, 6, 7, 0, 1, 2, 3]

_BUILT = None


def _build():
    import concourse.bass as bass
    import concourse.tile as tile
    from concourse import mybir, bacc

    f32 = mybir.dt.float32
    f32r = mybir.dt.float32r

    nc = bacc.Bacc("TRN2", target_bir_lowering=False, debug=False,
                   num_devices=NCORES)
    xT = nc.dram_tensor("xT", [D, TOK], f32r, kind="ExternalInput")
    wq = nc.dram_tensor("wq", [NKD, NKD, 128, 128], f32r, kind="ExternalInput")
    wk = nc.dram_tensor("wk", [NKD, NKD, 128, 128], f32r, kind="ExternalInput")
    wv = nc.dram_tensor("wv", [D, D], f32r, kind="ExternalInput")
    wo = nc.dram_tensor("wo", [128, NKD, D], f32r, kind="ExternalInput")
    mask = nc.dram_tensor("mask", [128, 3, 128], f32, kind="ExternalInput")
    vones = nc.dram_tensor("vones", [128, NHP], f32r, kind="ExternalInput")
    kbias = nc.dram_tensor("kbias", [128, NKD], f32, kind="ExternalInput")
    outT = nc.dram_tensor("outT", [D, CHUNK], f32, kind="ExternalOutput")
    # per-head softmax-recip row, bounced through DRAM to broadcast across
    # partitions (SBUF DMA sources cannot have a zero partition step)
    rscratch = nc.dram_tensor("rscratch", [H, CHUNK], f32, kind="Internal")

    x_view = xT.rearrange("(kd p) t -> p kd t", p=128)
    wv_r = wv.rearrange("(kd p) c -> p kd c", p=128)

    with tile.TileContext(nc) as tc:
        with tc.tile_pool(name="const", bufs=1) as constp, \
             tc.tile_pool(name="qkv", bufs=1) as qkvp, \
             tc.tile_pool(name="pt", bufs=3) as ptp:

            mask_sb = constp.tile([128, 3, 128], f32)
            nc.sync.dma_start(out=mask_sb, in_=mask[:, :, :])
            kbias_sb = constp.tile([128, NKD], f32)
            nc.sync.dma_start(out=kbias_sb, in_=kbias[:, :])
            ones_sb = constp.tile([128, NHP], f32r)
            nc.sync.dma_start(out=ones_sb, in_=vones[:, :])

            # persistent qkv buffers
            qT_sb = qkvp.tile([128, NHP, CHUNK], f32r)  # feature-major q
            kT_sb = qkvp.tile([128, NHP, TOK], f32r)    # feature-major k
            # v: per key-tile/pair/parity, 65 stationary cols [v(64), ones];
            # the ones column makes attV psum row 64 the softmax denominator
            v_sb = qkvp.tile([128, NKD, NHP, 2, 65], f32r)

            with tc.tile_pool(name="xp", bufs=1) as xp, \
                 tc.tile_pool(name="ps_p", bufs=4, space="PSUM") as ps_p:
                x_sb = xp.tile([128, NKD, TOK], f32r)
                nc.sync.dma_start(out=x_sb[:, :, CHUNK:TOK],
                                  in_=x_view[:, :, CHUNK:TOK])

                with tc.tile_pool(name="wkp", bufs=1) as wkp:
                    with tc.tile_pool(name="wqp", bufs=1) as wqp:
                        wq_t = [wqp.tile([128, NKD, 128], f32r,
                                         name=f"wq_{co}") for co in range(NKD)]
                        for co in range(NKD):
                            nc.sync.dma_start(
                                out=wq_t[co],
                                in_=wq.rearrange("co kd p c -> co p kd c")[co])
                        nc.sync.dma_start(out=x_sb[:, :, 0:CHUNK],
                                          in_=x_view[:, :, 0:CHUNK])
                        wk_t = [wkp.tile([128, NKD, 128], f32r,
                                         name=f"wk_{co}") for co in range(NKD)]
                        for co in range(NKD):
                            nc.sync.dma_start(
                                out=wk_t[co],
                                in_=wk.rearrange("co kd p c -> co p kd c")[co])

                        # ---- q projection (own tokens), feature-major
                        for co in range(NKD):
                            ps = ps_p.tile([128, CHUNK], f32, tag="ps")
                            for kd in range(NKD):
                                nc.tensor.matmul(
                                    ps[:], wq_t[co][:, kd, :],
                                    x_sb[:, kd, CHUNK:TOK],
                                    start=(kd == 0), stop=(kd == NKD - 1))
                            nc.vector.tensor_copy(qT_sb[:, co, :], ps[:])

                    # ---- k projection over all TOK keys (wv loads into the
                    # space wq frees)
                    with tc.tile_pool(name="wvp", bufs=1) as wvp:
                        wv_sb = wvp.tile([128, NKD, D], f32r)
                        nc.sync.dma_start(out=wv_sb, in_=wv_r)
                        for co in range(NKD):
                            for th in range(2):
                                ps = ps_p.tile([128, CHUNK], f32, tag="ps")
                                for kd in range(NKD):
                                    nc.tensor.matmul(
                                        ps[:], wk_t[co][:, kd, :],
                                        x_sb[:, kd, th * CHUNK:(th + 1) * CHUNK],
                                        start=(kd == 0), stop=(kd == NKD - 1))
                                nc.vector.tensor_copy(
                                    kT_sb[:, co, th * CHUNK:(th + 1) * CHUNK],
                                    ps[:])

                        # ---- v projection, token-major: keys on partitions
                        for tt in range(NKD):
                            for cv in range(2):
                                ps = ps_p.tile([128, CHUNK], f32, tag="ps")
                                for kd in range(NKD):
                                    nc.tensor.matmul(
                                        ps[:], x_sb[:, kd, tt * 128:(tt + 1) * 128],
                                        wv_sb[:, kd, cv * CHUNK:(cv + 1) * CHUNK],
                                        start=(kd == 0), stop=(kd == NKD - 1))
                                # scatter 8 heads (4 pairs) into the 65-col
                                # slots: even head -> cols 0:64, odd -> 1:65
                                ps4 = ps[:].rearrange(
                                    "p (g par d) -> p g par d", par=2, d=HD)
                                g0 = cv * 4
                                nc.vector.tensor_copy(
                                    v_sb[:, tt, g0:g0 + 4, 0, 0:HD],
                                    ps4[:, :, 0, :])
                                nc.vector.tensor_copy(
                                    v_sb[:, tt, g0:g0 + 4, 1, 0:HD],
                                    ps4[:, :, 1, :])
                        # ones columns: even parity col 64, odd parity col 0
                        for tt in range(NKD):
                            nc.vector.tensor_copy(
                                v_sb[:, tt, :, 0, HD], ones_sb[:, :])
                            nc.vector.tensor_copy(
                                v_sb[:, tt, :, 1, HD], ones_sb[:, :])

            # ---- attention (head pairs) + output projection
            with tc.tile_pool(name="attb", bufs=1) as attbp, \
                 tc.tile_pool(name="nrm", bufs=2) as nrmp, \
                 tc.tile_pool(name="oev", bufs=2) as oevp:

                # packed attT: partitions 0:64 = even head, 64:128 = odd head
                attT_sb = attbp.tile([128, NHP, CHUNK], f32r)
                wo_sb = attbp.tile([128, NKD, D], f32r)
                for g in range(NKD):
                    nc.scalar.dma_start(out=wo_sb[:, g, :], in_=wo[:, g, :])

                with tc.tile_pool(name="ps_s", bufs=2, space="PSUM") as ps_sc, \
                     tc.tile_pool(name="ps_at", bufs=2, space="PSUM") as ps_at:
                  for hp in range(NHP):
                    att_e = ps_at.tile([128, CHUNK], f32, tag="att_e")
                    att_o = ps_at.tile([128, CHUNK], f32, tag="att_o")
                    sc_tiles = {}
                    pt_tiles = {}

                    def emit_sc(i):
                        kb = KB_ORDER[i]
                        qlo, qhi = QRANGE[kb]
                        wdt = qhi - qlo
                        sc = ps_sc.tile([128, 2, CHUNK], f32, tag="sc")
                        sc_tiles[i] = sc
                        for s in range(2):
                            po = s * 64
                            nc.tensor.matmul(
                                sc[:, s, 0:wdt],
                                kT_sb[po:po + 64, hp, kb * 128:(kb + 1) * 128],
                                qT_sb[po:po + 64, hp, qlo:qhi],
                                start=True, stop=True)
                        # exp for both heads in one ACT instruction
                        pt = ptp.tile([128, 2, CHUNK], f32r, tag="pt")
                        pt_tiles[i] = pt
                        nc.scalar.activation(
                            pt[:, :, 0:wdt], sc[:, :, 0:wdt],
                            mybir.ActivationFunctionType.Exp,
                            bias=kbias_sb[:, kb:kb + 1], scale=SCALE)
                        # band-edge mask: one region per kb, both head slots
                        off, m0, m1 = MASKR[kb]
                        mw = (m1 - m0) * 128
                        msrc = mask_sb[:, m0:m1, :]
                        mbc = bass.AP(tensor=msrc.tensor, offset=msrc.offset,
                                      ap=[list(msrc.ap[0]), [0, 2]]
                                      + [list(a) for a in msrc.ap[1:]])
                        pslice = pt[:, :, off:off + mw]
                        pv = bass.AP(tensor=pslice.tensor, offset=pslice.offset,
                                     ap=[list(pslice.ap[0]), list(pslice.ap[1]),
                                         [128, mw // 128], [1, 128]])
                        nc.vector.tensor_mul(pv, pv, mbc)

                    def emit_att(i):
                        kb = KB_ORDER[i]
                        qlo, qhi = QRANGE[kb]
                        wdt = qhi - qlo
                        pt = pt_tiles.pop(i)
                        sc_tiles.pop(i)
                        nc.tensor.matmul(
                            att_e[0:65, qlo:qhi],
                            v_sb[:, kb, hp, 0, :],
                            pt[:, 0, 0:wdt],
                            start=(i == 0), stop=(i == len(KB_ORDER) - 1))
                        nc.tensor.matmul(
                            att_o[0:65, qlo:qhi],
                            v_sb[:, kb, hp, 1, :],
                            pt[:, 1, 0:wdt],
                            start=(i == 0), stop=(i == len(KB_ORDER) - 1))

                    # software-pipelined: run scores 2 kb-groups ahead
                    emit_sc(0)
                    emit_sc(1)
                    for i in range(8):
                        if i + 2 < 8:
                            emit_sc(i + 2)
                        emit_att(i)

                    # normalize: recip of sums row, DRAM-bounce broadcast.
                    # odd head lands in a staging tile, then a partition-
                    # shifting DMA moves it to attT partitions 64:128.
                    for s, att_ps in ((0, att_e), (1, att_o)):
                        h = 2 * hp + s
                        recip = nrmp.tile([128, CHUNK], f32, tag="recip")
                        nc.vector.reciprocal(recip[64:65, :],
                                             att_ps[64:65, :])
                        nc.sync.dma_start(out=rscratch[h:h + 1, :],
                                          in_=recip[64:65, :])
                        bc = nrmp.tile([128, CHUNK], f32, tag="bc")
                        bcast_src = bass.AP(
                            tensor=rscratch, offset=h * CHUNK,
                            ap=[[0, 64], [1, CHUNK]])
                        nc.gpsimd.dma_start(out=bc[0:64, :], in_=bcast_src)
                        if s == 0:
                            nc.vector.tensor_mul(
                                attT_sb[0:64, hp, :],
                                att_ps[0:64, :],
                                bc[0:64, :])
                        else:
                            stage = nrmp.tile([64, CHUNK], f32r, tag="stage")
                            nc.vector.tensor_mul(
                                stage[:, :], att_ps[0:64, :], bc[0:64, :])
                            nc.gpsimd.dma_start(
                                out=attT_sb[64:128, hp, :], in_=stage[:, :])

                # ---- output projection: K=128 per head pair
                with tc.tile_pool(name="ps_o", bufs=2, space="PSUM") as ps_o:
                  for eo in range(NKD):
                    ps = ps_o.tile([128, CHUNK], f32, tag="ps")
                    for hp in range(NHP):
                        nc.tensor.matmul(
                            ps[:],
                            wo_sb[:, hp, eo * 128:(eo + 1) * 128],
                            attT_sb[:, hp, :],
                            start=(hp == 0), stop=(hp == NHP - 1))
                    ot = oevp.tile([128, CHUNK], f32, tag="ot")
                    nc.vector.tensor_copy(ot[:], ps[:])
                    nc.scalar.dma_start(out=outT[eo * 128:(eo + 1) * 128, :],
                                        in_=ot[:])

    nc.compile()
    return nc


def _host_inputs(x, w_qkv, w_out):
    x = np.ascontiguousarray(np.asarray(x, dtype=np.float32))
    w_qkv = np.ascontiguousarray(np.asarray(w_qkv, dtype=np.float32))
    w_out = np.ascontiguousarray(np.asarray(w_out, dtype=np.float32))

    wq = w_qkv[:, 0:D]
    wk = w_qkv[:, D:2 * D]
    wv = np.ascontiguousarray(w_qkv[:, 2 * D:3 * D])

    # [co, kd, p, c] layout for per-co-tile streaming loads
    def co_kd(w):
        return np.ascontiguousarray(
            w.reshape(NKD, 128, NKD, 128).transpose(2, 0, 1, 3))

    wq_t, wk_t = co_kd(wq), co_kd(wk)

    # wo pair-major: partitions 0:64 = rows of head 2hp, 64:128 = head 2hp+1
    wo_t = np.ascontiguousarray(
        w_out.reshape(NHP, 2, HD, D).transpose(1, 2, 0, 3).reshape(128, NHP, D))

    r = np.arange(128)[:, None]
    c = np.arange(128)[None, :]
    mask = np.zeros((128, 3, 128), dtype=np.float32)
    mask[:, 0, :] = (r > c).astype(np.float32)
    mask[:, 2, :] = (r <= c).astype(np.float32)
    vones = np.ones((128, NHP), dtype=np.float32)

    in_maps = []
    for core in range(NCORES):
        b, qc = divmod(core, 4)
        q0 = qc * CHUNK
        xa = np.zeros((TOK, D), dtype=np.float32)
        lo = max(0, q0 - CHUNK)
        xa[CHUNK - (q0 - lo):] = x[b, lo:q0 + CHUNK]
        kb_bias = np.zeros((128, NKD), dtype=np.float32)
        if qc == 0:
            kb_bias[:, 0:4] = -250.0
        in_maps.append({
            "xT": np.ascontiguousarray(xa.T),
            "wq": wq_t, "wk": wk_t, "wv": wv, "wo": wo_t,
            "mask": mask, "kbias": kb_bias, "vones": vones,
        })
    return in_maps


def kernel(x, w_qkv, w_out):
    global _BUILT
    if _BUILT is None:
        _BUILT = _build()
    from concourse.bass_utils import run_bass_kernel_spmd

    in_maps = _host_inputs(x, w_qkv, w_out)
    res = run_bass_kernel_spmd(_BUILT, in_maps, core_ids=list(range(NCORES)))
    out = np.empty((B, T, D), dtype=np.float32)
    for core in range(NCORES):
        b, qc = divmod(core, 4)
        out[b, qc * CHUNK:(qc + 1) * CHUNK, :] = res.results[core]["outT"].T
    return out


# revision 6
# speedup vs baseline: 1.2151x; 1.0713x over previous
"""Causal sliding-window attention (B=2, T=2048, D=1024, H=16, W=512) on 8 trn2 cores.

Sequence-parallel: each core owns 512 tokens of one batch, recomputes the
512-token halo k/v locally. Head-paired attention: heads (2hp, 2hp+1) share
kT/qT partition halves; scores for both heads of a pair land in one 2-bank
psum tile so exp processes both at once (half the ACT fixed cost). attV for
the even head accumulates att rows 0..63 + softmax-sum row 64; the odd head
accumulates sum row 63 + att rows 64..127, so the packed attT tile feeds a
K=128 output projection against a host-prearranged pair-major wo.
Projection psum evictions ride DVE; normalization recip broadcasts bounce
through DRAM on the pool queue.
"""
import sys

sys.path.insert(0, "/opt/trn_rl_repo")

import numpy as np

B, T, D = 2, 2048, 1024
H, HD, W = 16, 64, 512
NCORES = 8
CHUNK = 512  # own tokens per core
TOK = 2 * CHUNK  # halo + own
NKD = D // 128  # 8 contraction tiles
NHP = H // 2  # head pairs
SCALE = HD ** -0.5

# query-window [qlo, qhi) per key-tile kb, padded to >=256 cols for fp32r rate
QRANGE = []
for kb in range(8):
    qlo = max(0, 128 * kb - 512)
    qhi = min(512, 128 * kb + 128)
    if qhi - qlo < 256:
        qlo, qhi = (0, 256) if qlo == 0 else (256, 512)
    QRANGE.append((qlo, qhi))

# per kb: one contiguous masked region (col offset rel. qlo, mask slots a:b)
# mask slots: 0 = strict-lower (j>q edge), 1 = zeros, 2 = upper-incl (far edge)
MASKR = {
    0: (0, 0, 2),    # [lower | zeros] over cols 0:256
    1: (128, 0, 1),
    2: (256, 0, 1),
    3: (384, 0, 1),
    4: (0, 2, 3),
    5: (0, 2, 3),
    6: (0, 2, 3),
    7: (0, 1, 3),    # [zeros | upper] over cols 0:256
}

# start with a full-width kb so the psum accumulation init covers all cols
KB_ORDER = [4, 5, 6, 7, 0, 1, 2, 3]

_BUILT = None


def _build():
    import concourse.bass as bass
    import concourse.tile as tile
    from concourse import mybir, bacc

    f32 = mybir.dt.float32
    f32r = mybir.dt.float32r

    nc = bacc.Bacc("TRN2", target_bir_lowering=False, debug=False,
                   num_devices=NCORES)
    xT = nc.dram_tensor("xT", [D, TOK], f32r, kind="ExternalInput")
    wq = nc.dram_tensor("wq", [NKD, NKD, 128, 128], f32r, kind="ExternalInput")
    wk = nc.dram_tensor("wk", [NKD, NKD, 128, 128], f32r, kind="ExternalInput")
    wv = nc.dram_tensor("wv", [D, D], f32r, kind="ExternalInput")
    wo = nc.dram_tensor("wo", [128, NKD, D], f32r, kind="ExternalInput")
    mask = nc.dram_tensor("mask", [128, 3, 128], f32, kind="ExternalInput")
    vones = nc.dram_tensor("vones", [128, NHP], f32r, kind="ExternalInput")
    vones64 = nc.dram_tensor("vones64", [128, HD], f32r, kind="ExternalInput")
    kbias = nc.dram_tensor("kbias", [128, NKD], f32, kind="ExternalInput")
    outT = nc.dram_tensor("outT", [D, CHUNK], f32, kind="ExternalOutput")

    x_view = xT.rearrange("(kd p) t -> p kd t", p=128)
    wv_r = wv.rearrange("(kd p) c -> p kd c", p=128)

    with tile.TileContext(nc) as tc:
        with tc.tile_pool(name="const", bufs=1) as constp, \
             tc.tile_pool(name="qkv", bufs=1) as qkvp, \
             tc.tile_pool(name="pt", bufs=3) as ptp:

            mask_sb = constp.tile([128, 3, 128], f32)
            kbias_sb = constp.tile([128, NKD], f32)
            nc.sync.dma_start(out=kbias_sb, in_=kbias[:, :])
            ones_sb = constp.tile([128, NHP], f32r)
            nc.sync.dma_start(out=ones_sb, in_=vones[:, :])
            ones64 = constp.tile([128, HD], f32r)
            nc.sync.dma_start(out=ones64, in_=vones64[:, :])

            # persistent qkv buffers
            qT_sb = qkvp.tile([128, NHP, CHUNK], f32r)  # feature-major q
            kT_sb = qkvp.tile([128, NHP, TOK], f32r)    # feature-major k
            # v: per key-tile/pair/parity, 65 stationary cols [v(64), ones];
            # the ones column makes attV psum row 64 the softmax denominator
            v_sb = qkvp.tile([128, NKD, NHP, 2, 65], f32r)

            with tc.tile_pool(name="xp", bufs=1) as xp, \
                 tc.tile_pool(name="ps_p", bufs=4, space="PSUM") as ps_p:
                x_sb = xp.tile([128, NKD, TOK], f32r)
                nc.sync.dma_start(out=x_sb[:, :, CHUNK:TOK],
                                  in_=x_view[:, :, CHUNK:TOK])
                nc.sync.dma_start(out=mask_sb, in_=mask[:, :, :])

                with tc.tile_pool(name="wkp", bufs=1) as wkp:
                    with tc.tile_pool(name="wqp", bufs=1) as wqp:
                        wq_t = [wqp.tile([128, NKD, 128], f32r,
                                         name=f"wq_{co}") for co in range(NKD)]
                        for co in range(NKD):
                            nc.sync.dma_start(
                                out=wq_t[co],
                                in_=wq.rearrange("co kd p c -> co p kd c")[co])
                        nc.sync.dma_start(out=x_sb[:, :, 0:CHUNK],
                                          in_=x_view[:, :, 0:CHUNK])
                        wk_t = [wkp.tile([128, NKD, 128], f32r,
                                         name=f"wk_{co}") for co in range(NKD)]
                        for co in range(NKD):
                            nc.sync.dma_start(
                                out=wk_t[co],
                                in_=wk.rearrange("co kd p c -> co p kd c")[co])

                        # ---- q projection (own tokens), feature-major
                        for co in range(NKD):
                            ps = ps_p.tile([128, CHUNK], f32, tag="ps")
                            for kd in range(NKD):
                                nc.tensor.matmul(
                                    ps[:], wq_t[co][:, kd, :],
                                    x_sb[:, kd, CHUNK:TOK],
                                    start=(kd == 0), stop=(kd == NKD - 1))
                            nc.scalar.copy(qT_sb[:, co, :], ps[:])

                    # ---- k projection over all TOK keys (wv loads into the
                    # space wq frees)
                    with tc.tile_pool(name="wvp", bufs=1) as wvp:
                        wv_sb = wvp.tile([128, NKD, D], f32r)
                        nc.sync.dma_start(out=wv_sb, in_=wv_r)
                        for co in range(NKD):
                            for th in range(2):
                                ps = ps_p.tile([128, CHUNK], f32, tag="ps")
                                for kd in range(NKD):
                                    nc.tensor.matmul(
                                        ps[:], wk_t[co][:, kd, :],
                                        x_sb[:, kd, th * CHUNK:(th + 1) * CHUNK],
                                        start=(kd == 0), stop=(kd == NKD - 1))
                                nc.scalar.copy(
                                    kT_sb[:, co, th * CHUNK:(th + 1) * CHUNK],
                                    ps[:])

                        # ---- v projection, token-major: keys on partitions
                        for tt in range(NKD):
                            for cv in range(2):
                                ps = ps_p.tile([128, CHUNK], f32, tag="ps")
                                for kd in range(NKD):
                                    nc.tensor.matmul(
                                        ps[:], x_sb[:, kd, tt * 128:(tt + 1) * 128],
                                        wv_sb[:, kd, cv * CHUNK:(cv + 1) * CHUNK],
                                        start=(kd == 0), stop=(kd == NKD - 1))
                                # scatter 8 heads (4 pairs) into the 65-col
                                # slots: even head -> cols 0:64, odd -> 1:65
                                ps4 = ps[:].rearrange(
                                    "p (g par d) -> p g par d", par=2, d=HD)
                                g0 = cv * 4
                                nc.scalar.copy(
                                    v_sb[:, tt, g0:g0 + 4, 0, 0:HD],
                                    ps4[:, :, 0, :])
                                nc.scalar.copy(
                                    v_sb[:, tt, g0:g0 + 4, 1, 0:HD],
                                    ps4[:, :, 1, :])
                        # ones columns: even parity col 64, odd parity col 0
                        for tt in range(NKD):
                            nc.scalar.copy(v_sb[:, tt, :, 0, HD],
                                           ones_sb[:, 0:NHP])
                            nc.scalar.copy(v_sb[:, tt, :, 1, HD],
                                           ones_sb[:, 0:NHP])

            # ---- attention (head pairs) + output projection
            with tc.tile_pool(name="attb", bufs=1) as attbp, \
                 tc.tile_pool(name="nrm", bufs=2) as nrmp, \
                 tc.tile_pool(name="oev", bufs=2) as oevp:

                # packed attT: partitions 0:64 = even head, 64:128 = odd head
                attT_sb = attbp.tile([128, NHP, CHUNK], f32r)
                wo_sb = attbp.tile([128, NKD, D], f32r)
                for g in range(NKD):
                    nc.scalar.dma_start(out=wo_sb[:, g, :], in_=wo[:, g, :])

                with tc.tile_pool(name="ps_s", bufs=2, space="PSUM") as ps_sc, \
                     tc.tile_pool(name="ps_at", bufs=3, space="PSUM") as ps_at, \
                     tc.tile_pool(name="ps_bc", bufs=1, space="PSUM") as ps_bc:
                  for hp in range(NHP):
                    att_e = ps_at.tile([128, CHUNK], f32, tag="att")
                    att_o = ps_at.tile([128, CHUNK], f32, tag="att")
                    sc_tiles = {}
                    pt_tiles = {}

                    def emit_sc(i):
                        kb = KB_ORDER[i]
                        qlo, qhi = QRANGE[kb]
                        wdt = qhi - qlo
                        sc = ps_sc.tile([128, 2, CHUNK], f32, tag="sc")
                        sc_tiles[i] = sc
                        for s in range(2):
                            po = s * 64
                            nc.tensor.matmul(
                                sc[:, s, 0:wdt],
                                kT_sb[po:po + 64, hp, kb * 128:(kb + 1) * 128],
                                qT_sb[po:po + 64, hp, qlo:qhi],
                                start=True, stop=True)
                        # exp for both heads in one ACT instruction
                        pt = ptp.tile([128, 2, CHUNK], f32r, tag="pt")
                        pt_tiles[i] = pt
                        nc.scalar.activation(
                            pt[:, :, 0:wdt], sc[:, :, 0:wdt],
                            mybir.ActivationFunctionType.Exp,
                            bias=kbias_sb[:, kb:kb + 1], scale=SCALE)
                        # band-edge mask: one region per kb, both head slots
                        off, m0, m1 = MASKR[kb]
                        mw = (m1 - m0) * 128
                        msrc = mask_sb[:, m0:m1, :]
                        mbc = bass.AP(tensor=msrc.tensor, offset=msrc.offset,
                                      ap=[list(msrc.ap[0]), [0, 2]]
                                      + [list(a) for a in msrc.ap[1:]])
                        pslice = pt[:, :, off:off + mw]
                        pv = bass.AP(tensor=pslice.tensor, offset=pslice.offset,
                                     ap=[list(pslice.ap[0]), list(pslice.ap[1]),
                                         [128, mw // 128], [1, 128]])
                        nc.vector.tensor_mul(pv, pv, mbc)

                    def emit_att(i):
                        kb = KB_ORDER[i]
                        qlo, qhi = QRANGE[kb]
                        wdt = qhi - qlo
                        pt = pt_tiles.pop(i)
                        sc_tiles.pop(i)
                        nc.tensor.matmul(
                            att_e[0:65, qlo:qhi],
                            v_sb[:, kb, hp, 0, :],
                            pt[:, 0, 0:wdt],
                            start=(i == 0), stop=(i == len(KB_ORDER) - 1))
                        nc.tensor.matmul(
                            att_o[0:65, qlo:qhi],
                            v_sb[:, kb, hp, 1, :],
                            pt[:, 1, 0:wdt],
                            start=(i == 0), stop=(i == len(KB_ORDER) - 1))

                    # software-pipelined: run scores 2 kb-groups ahead
                    emit_sc(0)
                    emit_sc(1)
                    for i in range(8):
                        if i + 2 < 8:
                            emit_sc(i + 2)
                        emit_att(i)

                    # normalize: reciprocal of the sums row, broadcast to 64
                    # partitions with a PE rank-1 outer product (ones x recip),
                    # then psum-evicting multiply. The odd head goes through a
                    # staging tile + partition-shifting DMA to attT rows 64:128.
                    for s, att_ps in ((0, att_e), (1, att_o)):
                        recip = nrmp.tile([128, CHUNK], f32r, tag="recip")
                        with nc.allow_low_precision(
                                reason="f32r recip row is bit-identical f32"):
                            nc.vector.reciprocal(recip[64:65, :],
                                                 att_ps[64:65, :])
                        bc = ps_bc.tile([128, CHUNK], f32, tag="bc")
                        nc.tensor.matmul(bc[0:64, :], ones64[64:65, :],
                                         recip[64:65, :],
                                         start=True, stop=True)
                        if s == 0:
                            nc.vector.tensor_mul(
                                attT_sb[0:64, hp, :],
                                att_ps[0:64, :],
                                bc[0:64, :])
                        else:
                            stage = nrmp.tile([64, CHUNK], f32r, tag="stage")
                            nc.vector.tensor_mul(
                                stage[:, :], att_ps[0:64, :], bc[0:64, :])
                            nc.sync.dma_start(
                                out=attT_sb[64:128, hp, :], in_=stage[:, :])

                # ---- output projection: K=128 per head pair
                with tc.tile_pool(name="ps_o", bufs=2, space="PSUM") as ps_o:
                  for eo in range(NKD):
                    ps = ps_o.tile([128, CHUNK], f32, tag="ps")
                    for hp in range(NHP):
                        nc.tensor.matmul(
                            ps[:],
                            wo_sb[:, hp, eo * 128:(eo + 1) * 128],
                            attT_sb[:, hp, :],
                            start=(hp == 0), stop=(hp == NHP - 1))
                    ot = oevp.tile([128, CHUNK], f32, tag="ot")
                    nc.scalar.copy(ot[:], ps[:])
                    nc.scalar.dma_start(out=outT[eo * 128:(eo + 1) * 128, :],
                                        in_=ot[:])

    nc.compile()
    return nc


def _host_inputs(x, w_qkv, w_out):
    x = np.ascontiguousarray(np.asarray(x, dtype=np.float32))
    w_qkv = np.ascontiguousarray(np.asarray(w_qkv, dtype=np.float32))
    w_out = np.ascontiguousarray(np.asarray(w_out, dtype=np.float32))

    wq = w_qkv[:, 0:D]
    wk = w_qkv[:, D:2 * D]
    wv = np.ascontiguousarray(w_qkv[:, 2 * D:3 * D])

    # [co, kd, p, c] layout for per-co-tile streaming loads
    def co_kd(w):
        return np.ascontiguousarray(
            w.reshape(NKD, 128, NKD, 128).transpose(2, 0, 1, 3))

    wq_t, wk_t = co_kd(wq), co_kd(wk)

    # wo pair-major: partitions 0:64 = rows of head 2hp, 64:128 = head 2hp+1
    wo_t = np.ascontiguousarray(
        w_out.reshape(NHP, 2, HD, D).transpose(1, 2, 0, 3).reshape(128, NHP, D))

    r = np.arange(128)[:, None]
    c = np.arange(128)[None, :]
    mask = np.zeros((128, 3, 128), dtype=np.float32)
    mask[:, 0, :] = (r > c).astype(np.float32)
    mask[:, 2, :] = (r <= c).astype(np.float32)
    vones = np.ones((128, NHP), dtype=np.float32)
    vones64 = np.ones((128, HD), dtype=np.float32)

    in_maps = []
    for core in range(NCORES):
        b, qc = divmod(core, 4)
        q0 = qc * CHUNK
        xa = np.zeros((TOK, D), dtype=np.float32)
        lo = max(0, q0 - CHUNK)
        xa[CHUNK - (q0 - lo):] = x[b, lo:q0 + CHUNK]
        kb_bias = np.zeros((128, NKD), dtype=np.float32)
        if qc == 0:
            kb_bias[:, 0:4] = -250.0
        in_maps.append({
            "xT": np.ascontiguousarray(xa.T),
            "wq": wq_t, "wk": wk_t, "wv": wv, "wo": wo_t,
            "mask": mask, "kbias": kb_bias, "vones": vones,
            "vones64": vones64,
        })
    return in_maps


def kernel(x, w_qkv, w_out):
    global _BUILT
    if _BUILT is None:
        _BUILT = _build()
    from concourse.bass_utils import run_bass_kernel_spmd

    in_maps = _host_inputs(x, w_qkv, w_out)
    res = run_bass_kernel_spmd(_BUILT, in_maps, core_ids=list(range(NCORES)))
    out = np.empty((B, T, D), dtype=np.float32)
    for core in range(NCORES):
        b, qc = divmod(core, 4)
        out[b, qc * CHUNK:(qc + 1) * CHUNK, :] = res.results[core]["outT"].T
    return out


# revision 8
# speedup vs baseline: 1.2190x; 1.0032x over previous
"""Causal sliding-window attention (B=2, T=2048, D=1024, H=16, W=512) on 8 trn2 cores.

Sequence-parallel: each core owns 512 tokens of one batch, recomputes the
512-token halo k/v locally. Head-paired attention: heads (2hp, 2hp+1) share
kT/qT partition halves; scores for both heads of a pair land in one 2-bank
psum tile so exp processes both at once (half the ACT fixed cost). attV for
the even head accumulates att rows 0..63 + softmax-sum row 64; the odd head
accumulates sum row 63 + att rows 64..127, so the packed attT tile feeds a
K=128 output projection against a host-prearranged pair-major wo.
Projection psum evictions ride DVE; normalization recip broadcasts bounce
through DRAM on the pool queue.
"""
import sys

sys.path.insert(0, "/opt/trn_rl_repo")

import numpy as np

B, T, D = 2, 2048, 1024
H, HD, W = 16, 64, 512
NCORES = 8
CHUNK = 512  # own tokens per core
TOK = 2 * CHUNK  # halo + own
NKD = D // 128  # 8 contraction tiles
NHP = H // 2  # head pairs
SCALE = HD ** -0.5

# query-window [qlo, qhi) per key-tile kb, padded to >=256 cols for fp32r rate
QRANGE = []
for kb in range(8):
    qlo = max(0, 128 * kb - 512)
    qhi = min(512, 128 * kb + 128)
    if qhi - qlo < 256:
        qlo, qhi = (0, 256) if qlo == 0 else (256, 512)
    QRANGE.append((qlo, qhi))

# per kb: one contiguous masked region (col offset rel. qlo, mask slots a:b)
# mask slots: 0 = strict-lower (j>q edge), 1 = zeros, 2 = upper-incl (far edge)
MASKR = {
    0: (0, 0, 2),    # [lower | zeros] over cols 0:256
    1: (128, 0, 1),
    2: (256, 0, 1),
    3: (384, 0, 1),
    4: (0, 2, 3),
    5: (0, 2, 3),
    6: (0, 2, 3),
    7: (0, 1, 3),    # [zeros | upper] over cols 0:256
}

# start with a full-width kb so the psum accumulation init covers all cols
KB_ORDER = [4, 5, 6, 7, 0, 1, 2, 3]

_BUILT = None


def _build():
    import concourse.bass as bass
    import concourse.tile as tile
    from concourse import mybir, bacc

    f32 = mybir.dt.float32
    f32r = mybir.dt.float32r

    nc = bacc.Bacc("TRN2", target_bir_lowering=False, debug=False,
                   num_devices=NCORES)
    xT = nc.dram_tensor("xT", [D, TOK], f32r, kind="ExternalInput")
    wq = nc.dram_tensor("wq", [NKD, NKD, 128, 128], f32r, kind="ExternalInput")
    wk = nc.dram_tensor("wk", [NKD, NKD, 128, 128], f32r, kind="ExternalInput")
    wv = nc.dram_tensor("wv", [D, D], f32r, kind="ExternalInput")
    wo = nc.dram_tensor("wo", [128, NKD, D], f32r, kind="ExternalInput")
    mask = nc.dram_tensor("mask", [128, 3, 128], f32, kind="ExternalInput")
    vones = nc.dram_tensor("vones", [128, NHP], f32r, kind="ExternalInput")
    vones64 = nc.dram_tensor("vones64", [128, HD], f32r, kind="ExternalInput")
    kbias = nc.dram_tensor("kbias", [128, NKD], f32, kind="ExternalInput")
    outT = nc.dram_tensor("outT", [D, CHUNK], f32, kind="ExternalOutput")

    x_view = xT.rearrange("(kd p) t -> p kd t", p=128)
    wv_r = wv.rearrange("(kd p) c -> p kd c", p=128)

    with tile.TileContext(nc) as tc:
        with tc.tile_pool(name="const", bufs=1) as constp, \
             tc.tile_pool(name="qkv", bufs=1) as qkvp, \
             tc.tile_pool(name="pt", bufs=3) as ptp:

            mask_sb = constp.tile([128, 3, 128], f32)
            kbias_sb = constp.tile([128, NKD], f32)
            ones_sb = constp.tile([128, NHP], f32r)
            ones64 = constp.tile([128, HD], f32r)

            # persistent qkv buffers
            qT_sb = qkvp.tile([128, NHP, CHUNK], f32r)  # feature-major q
            kT_sb = qkvp.tile([128, NHP, TOK], f32r)    # feature-major k
            # v: per key-tile/pair/parity, 65 stationary cols [v(64), ones];
            # the ones column makes attV psum row 64 the softmax denominator
            v_sb = qkvp.tile([128, NKD, NHP, 2, 65], f32r)

            with tc.tile_pool(name="xp", bufs=1) as xp, \
                 tc.tile_pool(name="ps_p", bufs=4, space="PSUM") as ps_p:
                x_sb = xp.tile([128, NKD, TOK], f32r)
                nc.sync.dma_start(out=x_sb[:, :, CHUNK:TOK],
                                  in_=x_view[:, :, CHUNK:TOK])

                with tc.tile_pool(name="wkp", bufs=1) as wkp:
                    with tc.tile_pool(name="wqp", bufs=1) as wqp:
                        wq_t = [wqp.tile([128, NKD, 128], f32r,
                                         name=f"wq_{co}") for co in range(NKD)]
                        for co in range(NKD):
                            nc.sync.dma_start(
                                out=wq_t[co],
                                in_=wq.rearrange("co kd p c -> co p kd c")[co])
                        nc.sync.dma_start(out=x_sb[:, :, 0:CHUNK],
                                          in_=x_view[:, :, 0:CHUNK])
                        nc.sync.dma_start(out=mask_sb, in_=mask[:, :, :])
                        nc.sync.dma_start(out=kbias_sb, in_=kbias[:, :])
                        nc.sync.dma_start(out=ones_sb, in_=vones[:, :])
                        nc.sync.dma_start(out=ones64, in_=vones64[:, :])
                        wk_t = [wkp.tile([128, NKD, 128], f32r,
                                         name=f"wk_{co}") for co in range(NKD)]
                        for co in range(NKD):
                            nc.sync.dma_start(
                                out=wk_t[co],
                                in_=wk.rearrange("co kd p c -> co p kd c")[co])

                        # ---- q projection (own tokens), feature-major
                        for co in range(NKD):
                            ps = ps_p.tile([128, CHUNK], f32, tag="ps")
                            for kd in range(NKD):
                                nc.tensor.matmul(
                                    ps[:], wq_t[co][:, kd, :],
                                    x_sb[:, kd, CHUNK:TOK],
                                    start=(kd == 0), stop=(kd == NKD - 1))
                            nc.scalar.copy(qT_sb[:, co, :], ps[:])

                    # ---- k projection over all TOK keys (wv loads into the
                    # space wq frees)
                    with tc.tile_pool(name="wvp", bufs=1) as wvp:
                        wv_sb = wvp.tile([128, NKD, D], f32r)
                        nc.sync.dma_start(out=wv_sb, in_=wv_r)
                        for co in range(NKD):
                            for th in range(2):
                                ps = ps_p.tile([128, CHUNK], f32, tag="ps")
                                for kd in range(NKD):
                                    nc.tensor.matmul(
                                        ps[:], wk_t[co][:, kd, :],
                                        x_sb[:, kd, th * CHUNK:(th + 1) * CHUNK],
                                        start=(kd == 0), stop=(kd == NKD - 1))
                                nc.scalar.copy(
                                    kT_sb[:, co, th * CHUNK:(th + 1) * CHUNK],
                                    ps[:])

                        # ---- v projection, token-major: keys on partitions
                        for tt in range(NKD):
                            for cv in range(2):
                                ps = ps_p.tile([128, CHUNK], f32, tag="ps")
                                for kd in range(NKD):
                                    nc.tensor.matmul(
                                        ps[:], x_sb[:, kd, tt * 128:(tt + 1) * 128],
                                        wv_sb[:, kd, cv * CHUNK:(cv + 1) * CHUNK],
                                        start=(kd == 0), stop=(kd == NKD - 1))
                                # scatter 8 heads (4 pairs) into the 65-col
                                # slots: even head -> cols 0:64, odd -> 1:65
                                ps4 = ps[:].rearrange(
                                    "p (g par d) -> p g par d", par=2, d=HD)
                                g0 = cv * 4
                                nc.scalar.copy(
                                    v_sb[:, tt, g0:g0 + 4, 0, 0:HD],
                                    ps4[:, :, 0, :])
                                nc.scalar.copy(
                                    v_sb[:, tt, g0:g0 + 4, 1, 0:HD],
                                    ps4[:, :, 1, :])
                        # ones columns: even parity col 64, odd parity col 0
                        for tt in range(NKD):
                            nc.scalar.copy(v_sb[:, tt, :, 0, HD],
                                           ones_sb[:, 0:NHP])
                            nc.scalar.copy(v_sb[:, tt, :, 1, HD],
                                           ones_sb[:, 0:NHP])

            # ---- attention (head pairs) + output projection
            with tc.tile_pool(name="attb", bufs=1) as attbp, \
                 tc.tile_pool(name="nrm", bufs=2) as nrmp, \
                 tc.tile_pool(name="oev", bufs=2) as oevp:

                # packed attT: partitions 0:64 = even head, 64:128 = odd head
                attT_sb = attbp.tile([128, NHP, CHUNK], f32r)
                wo_sb = attbp.tile([128, NKD, D], f32r)
                for g in range(NKD):
                    nc.scalar.dma_start(out=wo_sb[:, g, :], in_=wo[:, g, :])

                with tc.tile_pool(name="ps_s", bufs=2, space="PSUM") as ps_sc, \
                     tc.tile_pool(name="ps_at", bufs=3, space="PSUM") as ps_at, \
                     tc.tile_pool(name="ps_bc", bufs=1, space="PSUM") as ps_bc:
                  pending_norm = [None]
                  for hp in range(NHP):
                    att_e = ps_at.tile([128, CHUNK], f32, tag="att")
                    att_o = ps_at.tile([128, CHUNK], f32, tag="att")
                    sc_tiles = {}
                    pt_tiles = {}

                    def emit_sc(i):
                        kb = KB_ORDER[i]
                        qlo, qhi = QRANGE[kb]
                        wdt = qhi - qlo
                        sc = ps_sc.tile([128, 2, CHUNK], f32, tag="sc")
                        sc_tiles[i] = sc
                        for s in range(2):
                            po = s * 64
                            nc.tensor.matmul(
                                sc[:, s, 0:wdt],
                                kT_sb[po:po + 64, hp, kb * 128:(kb + 1) * 128],
                                qT_sb[po:po + 64, hp, qlo:qhi],
                                start=True, stop=True)
                        # exp for both heads in one ACT instruction
                        pt = ptp.tile([128, 2, CHUNK], f32r, tag="pt")
                        pt_tiles[i] = pt
                        nc.scalar.activation(
                            pt[:, :, 0:wdt], sc[:, :, 0:wdt],
                            mybir.ActivationFunctionType.Exp,
                            bias=kbias_sb[:, kb:kb + 1], scale=SCALE)
                        # band-edge mask: one region per kb, both head slots
                        off, m0, m1 = MASKR[kb]
                        mw = (m1 - m0) * 128
                        msrc = mask_sb[:, m0:m1, :]
                        mbc = bass.AP(tensor=msrc.tensor, offset=msrc.offset,
                                      ap=[list(msrc.ap[0]), [0, 2]]
                                      + [list(a) for a in msrc.ap[1:]])
                        pslice = pt[:, :, off:off + mw]
                        pv = bass.AP(tensor=pslice.tensor, offset=pslice.offset,
                                     ap=[list(pslice.ap[0]), list(pslice.ap[1]),
                                         [128, mw // 128], [1, 128]])
                        nc.vector.tensor_mul(pv, pv, mbc)

                    def emit_att(i):
                        kb = KB_ORDER[i]
                        qlo, qhi = QRANGE[kb]
                        wdt = qhi - qlo
                        pt = pt_tiles.pop(i)
                        sc_tiles.pop(i)
                        nc.tensor.matmul(
                            att_e[0:65, qlo:qhi],
                            v_sb[:, kb, hp, 0, :],
                            pt[:, 0, 0:wdt],
                            start=(i == 0), stop=(i == len(KB_ORDER) - 1))
                        nc.tensor.matmul(
                            att_o[0:65, qlo:qhi],
                            v_sb[:, kb, hp, 1, :],
                            pt[:, 1, 0:wdt],
                            start=(i == 0), stop=(i == len(KB_ORDER) - 1))

                    # software-pipelined: run scores 2 kb-groups ahead; the
                    # previous pair's normalize is emitted after this pair's
                    # first scores so PE never stalls on the reciprocal.
                    emit_sc(0)
                    emit_sc(1)
                    if pending_norm[0] is not None:
                        pending_norm[0]()
                    for i in range(8):
                        if i + 2 < 8:
                            emit_sc(i + 2)
                        emit_att(i)

                    # normalize: reciprocal of the sums row (issued now, off
                    # PE), then deferred: broadcast to 64 partitions with a PE
                    # rank-1 outer product (ones x recip) and psum-evicting
                    # multiply. The odd head goes through a staging tile +
                    # partition-shifting DMA to attT rows 64:128.
                    recips = []
                    for s, att_ps in ((0, att_e), (1, att_o)):
                        recip = nrmp.tile([128, CHUNK], f32r, tag="recip")
                        with nc.allow_low_precision(
                                reason="f32r recip row is bit-identical f32"):
                            nc.vector.reciprocal(recip[64:65, :],
                                                 att_ps[64:65, :])
                        recips.append(recip)

                    def norm_closure(hp=hp, att_e=att_e, att_o=att_o,
                                     recips=recips):
                        for s, att_ps, recip in ((0, att_e, recips[0]),
                                                 (1, att_o, recips[1])):
                            bc = ps_bc.tile([128, CHUNK], f32, tag="bc")
                            nc.tensor.matmul(bc[0:64, :], ones64[64:65, :],
                                             recip[64:65, :],
                                             start=True, stop=True)
                            if s == 0:
                                nc.vector.tensor_mul(
                                    attT_sb[0:64, hp, :],
                                    att_ps[0:64, :],
                                    bc[0:64, :])
                            else:
                                stage = nrmp.tile([64, CHUNK], f32r,
                                                  tag="stage")
                                nc.vector.tensor_mul(
                                    stage[:, :], att_ps[0:64, :], bc[0:64, :])
                                nc.sync.dma_start(
                                    out=attT_sb[64:128, hp, :],
                                    in_=stage[:, :])

                    pending_norm[0] = norm_closure
                  pending_norm[0]()

                # ---- output projection: K=128 per head pair
                with tc.tile_pool(name="ps_o", bufs=2, space="PSUM") as ps_o:
                  for eo in range(NKD):
                    ps = ps_o.tile([128, CHUNK], f32, tag="ps")
                    for hp in range(NHP):
                        nc.tensor.matmul(
                            ps[:],
                            wo_sb[:, hp, eo * 128:(eo + 1) * 128],
                            attT_sb[:, hp, :],
                            start=(hp == 0), stop=(hp == NHP - 1))
                    ot = oevp.tile([128, CHUNK], f32, tag="ot")
                    nc.scalar.copy(ot[:], ps[:])
                    nc.scalar.dma_start(out=outT[eo * 128:(eo + 1) * 128, :],
                                        in_=ot[:])

    nc.compile()
    return nc


def _host_inputs(x, w_qkv, w_out):
    x = np.ascontiguousarray(np.asarray(x, dtype=np.float32))
    w_qkv = np.ascontiguousarray(np.asarray(w_qkv, dtype=np.float32))
    w_out = np.ascontiguousarray(np.asarray(w_out, dtype=np.float32))

    wq = w_qkv[:, 0:D]
    wk = w_qkv[:, D:2 * D]
    wv = np.ascontiguousarray(w_qkv[:, 2 * D:3 * D])

    # [co, kd, p, c] layout for per-co-tile streaming loads
    def co_kd(w):
        return np.ascontiguousarray(
            w.reshape(NKD, 128, NKD, 128).transpose(2, 0, 1, 3))

    wq_t, wk_t = co_kd(wq), co_kd(wk)

    # wo pair-major: partitions 0:64 = rows of head 2hp, 64:128 = head 2hp+1
    wo_t = np.ascontiguousarray(
        w_out.reshape(NHP, 2, HD, D).transpose(1, 2, 0, 3).reshape(128, NHP, D))

    r = np.arange(128)[:, None]
    c = np.arange(128)[None, :]
    mask = np.zeros((128, 3, 128), dtype=np.float32)
    mask[:, 0, :] = (r > c).astype(np.float32)
    mask[:, 2, :] = (r <= c).astype(np.float32)
    vones = np.ones((128, NHP), dtype=np.float32)
    vones64 = np.ones((128, HD), dtype=np.float32)

    in_maps = []
    for core in range(NCORES):
        b, qc = divmod(core, 4)
        q0 = qc * CHUNK
        xa = np.zeros((TOK, D), dtype=np.float32)
        lo = max(0, q0 - CHUNK)
        xa[CHUNK - (q0 - lo):] = x[b, lo:q0 + CHUNK]
        kb_bias = np.zeros((128, NKD), dtype=np.float32)
        if qc == 0:
            kb_bias[:, 0:4] = -250.0
        in_maps.append({
            "xT": np.ascontiguousarray(xa.T),
            "wq": wq_t, "wk": wk_t, "wv": wv, "wo": wo_t,
            "mask": mask, "kbias": kb_bias, "vones": vones,
            "vones64": vones64,
        })
    return in_maps


def kernel(x, w_qkv, w_out):
    global _BUILT
    if _BUILT is None:
        _BUILT = _build()
    from concourse.bass_utils import run_bass_kernel_spmd

    in_maps = _host_inputs(x, w_qkv, w_out)
    res = run_bass_kernel_spmd(_BUILT, in_maps, core_ids=list(range(NCORES)))
    out = np.empty((B, T, D), dtype=np.float32)
    for core in range(NCORES):
        b, qc = divmod(core, 4)
        out[b, qc * CHUNK:(qc + 1) * CHUNK, :] = res.results[core]["outT"].T
    return out


# revision 9
# speedup vs baseline: 1.2773x; 1.0478x over previous
"""Causal sliding-window attention (B=2, T=2048, D=1024, H=16, W=512) on 8 trn2 cores.

Sequence-parallel: each core owns 512 tokens of one batch, recomputes the
512-token halo k/v locally. Head-paired attention: heads (2hp, 2hp+1) share
kT/qT partition halves; scores for both heads of a pair land in one 2-bank
psum tile so exp processes both at once (half the ACT fixed cost). attV for
the even head accumulates att rows 0..63 + softmax-sum row 64; the odd head
accumulates sum row 63 + att rows 64..127, so the packed attT tile feeds a
K=128 output projection against a host-prearranged pair-major wo.
Projection psum evictions ride DVE; normalization recip broadcasts bounce
through DRAM on the pool queue.
"""
import sys

sys.path.insert(0, "/opt/trn_rl_repo")

import numpy as np

B, T, D = 2, 2048, 1024
H, HD, W = 16, 64, 512
NCORES = 8
CHUNK = 512  # own tokens per core
TOK = 2 * CHUNK  # halo + own
NKD = D // 128  # 8 contraction tiles
NHP = H // 2  # head pairs
SCALE = HD ** -0.5

# query-window [qlo, qhi) per key-tile kb, padded to >=256 cols for fp32r rate
QRANGE = []
for kb in range(8):
    qlo = max(0, 128 * kb - 512)
    qhi = min(512, 128 * kb + 128)
    if qhi - qlo < 256:
        qlo, qhi = (0, 256) if qlo == 0 else (256, 512)
    QRANGE.append((qlo, qhi))

# per kb: one contiguous masked region (col offset rel. qlo, mask slots a:b)
# mask slots: 0 = strict-lower (j>q edge), 1 = zeros, 2 = upper-incl (far edge)
MASKR = {
    0: (0, 0, 2),    # [lower | zeros] over cols 0:256
    1: (128, 0, 1),
    2: (256, 0, 1),
    3: (384, 0, 1),
    4: (0, 2, 3),
    5: (0, 2, 3),
    6: (0, 2, 3),
    7: (0, 1, 3),    # [zeros | upper] over cols 0:256
}

# start with a full-width kb so the psum accumulation init covers all cols
KB_ORDER = [4, 5, 6, 7, 0, 1, 2, 3]

_BUILT = None


def _build():
    import concourse.bass as bass
    import concourse.tile as tile
    from concourse import mybir, bacc

    f32 = mybir.dt.float32
    f32r = mybir.dt.float32r

    nc = bacc.Bacc("TRN2", target_bir_lowering=False, debug=False,
                   num_devices=NCORES)
    xT = nc.dram_tensor("xT", [D, TOK], f32r, kind="ExternalInput")
    wq = nc.dram_tensor("wq", [NKD, NKD, 128, 128], f32r, kind="ExternalInput")
    wk = nc.dram_tensor("wk", [NKD, NKD, 128, 128], f32r, kind="ExternalInput")
    wv = nc.dram_tensor("wv", [D, D], f32r, kind="ExternalInput")
    wo = nc.dram_tensor("wo", [128, NKD, D], f32r, kind="ExternalInput")
    mask = nc.dram_tensor("mask", [128, 3, 128], f32, kind="ExternalInput")
    vones = nc.dram_tensor("vones", [128, NHP], f32r, kind="ExternalInput")
    vones64 = nc.dram_tensor("vones64", [128, HD], f32r, kind="ExternalInput")
    kbias = nc.dram_tensor("kbias", [128, NKD], f32, kind="ExternalInput")
    outT = nc.dram_tensor("outT", [D, CHUNK], f32, kind="ExternalOutput")

    x_view = xT.rearrange("(kd p) t -> p kd t", p=128)
    wv_r = wv.rearrange("(kd p) c -> p kd c", p=128)

    with tile.TileContext(nc) as tc:
        with tc.tile_pool(name="const", bufs=1) as constp, \
             tc.tile_pool(name="qkv", bufs=1) as qkvp, \
             tc.tile_pool(name="pt", bufs=3) as ptp:

            mask_sb = constp.tile([128, 3, 128], f32)
            kbias_sb = constp.tile([128, NKD], f32)
            ones_sb = constp.tile([128, NHP], f32r)
            ones64 = constp.tile([128, HD], f32r)

            # persistent qkv buffers
            qT_sb = qkvp.tile([128, NHP, CHUNK], f32r)  # feature-major q
            kT_sb = qkvp.tile([128, NHP, TOK], f32r)    # feature-major k
            # v: per key-tile/pair/parity, 65 stationary cols [v(64), ones];
            # the ones column makes attV psum row 64 the softmax denominator
            v_sb = qkvp.tile([128, NKD, NHP, 2, 65], f32r)

            with tc.tile_pool(name="xp", bufs=1) as xp, \
                 tc.tile_pool(name="ps_p", bufs=4, space="PSUM") as ps_p:
                x_sb = xp.tile([128, NKD, TOK], f32r)
                nc.sync.dma_start(out=x_sb[:, :, CHUNK:TOK],
                                  in_=x_view[:, :, CHUNK:TOK])

                with tc.tile_pool(name="wkp", bufs=1) as wkp:
                    with tc.tile_pool(name="wqp", bufs=1) as wqp:
                        wq_t = [wqp.tile([128, NKD, 128], f32r,
                                         name=f"wq_{co}") for co in range(NKD)]
                        for co in range(NKD):
                            nc.sync.dma_start(
                                out=wq_t[co],
                                in_=wq.rearrange("co kd p c -> co p kd c")[co])
                        nc.sync.dma_start(out=x_sb[:, :, 0:CHUNK],
                                          in_=x_view[:, :, 0:CHUNK])
                        nc.sync.dma_start(out=mask_sb, in_=mask[:, :, :])
                        nc.sync.dma_start(out=kbias_sb, in_=kbias[:, :])
                        nc.sync.dma_start(out=ones_sb, in_=vones[:, :])
                        nc.sync.dma_start(out=ones64, in_=vones64[:, :])
                        wk_t = [wkp.tile([128, NKD, 128], f32r,
                                         name=f"wk_{co}") for co in range(NKD)]
                        for co in range(NKD):
                            nc.sync.dma_start(
                                out=wk_t[co],
                                in_=wk.rearrange("co kd p c -> co p kd c")[co])

                        # ---- q projection (own tokens), feature-major
                        for co in range(NKD):
                            ps = ps_p.tile([128, CHUNK], f32, tag="ps")
                            for kd in range(NKD):
                                nc.tensor.matmul(
                                    ps[:], wq_t[co][:, kd, :],
                                    x_sb[:, kd, CHUNK:TOK],
                                    start=(kd == 0), stop=(kd == NKD - 1))
                            nc.scalar.copy(qT_sb[:, co, :], ps[:])

                    # ---- k projection over all TOK keys (wv loads into the
                    # space wq frees)
                    with tc.tile_pool(name="wvp", bufs=1) as wvp:
                        wv_sb = wvp.tile([128, NKD, D], f32r)
                        nc.sync.dma_start(out=wv_sb, in_=wv_r)
                        for co in range(NKD):
                            for th in range(2):
                                ps = ps_p.tile([128, CHUNK], f32, tag="ps")
                                for kd in range(NKD):
                                    nc.tensor.matmul(
                                        ps[:], wk_t[co][:, kd, :],
                                        x_sb[:, kd, th * CHUNK:(th + 1) * CHUNK],
                                        start=(kd == 0), stop=(kd == NKD - 1))
                                nc.scalar.copy(
                                    kT_sb[:, co, th * CHUNK:(th + 1) * CHUNK],
                                    ps[:])

                        # ---- v projection, token-major: keys on partitions
                        for tt in range(NKD):
                            for cv in range(2):
                                ps = ps_p.tile([128, CHUNK], f32, tag="ps")
                                for kd in range(NKD):
                                    nc.tensor.matmul(
                                        ps[:], x_sb[:, kd, tt * 128:(tt + 1) * 128],
                                        wv_sb[:, kd, cv * CHUNK:(cv + 1) * CHUNK],
                                        start=(kd == 0), stop=(kd == NKD - 1))
                                # scatter 8 heads (4 pairs) into the 65-col
                                # slots: even head -> cols 0:64, odd -> 1:65
                                ps4 = ps[:].rearrange(
                                    "p (g par d) -> p g par d", par=2, d=HD)
                                g0 = cv * 4
                                nc.scalar.copy(
                                    v_sb[:, tt, g0:g0 + 4, 0, 0:HD],
                                    ps4[:, :, 0, :])
                                nc.scalar.copy(
                                    v_sb[:, tt, g0:g0 + 4, 1, 0:HD],
                                    ps4[:, :, 1, :])
                        # ones columns: even parity col 64, odd parity col 0
                        for tt in range(NKD):
                            nc.scalar.copy(v_sb[:, tt, :, 0, HD],
                                           ones_sb[:, 0:NHP])
                            nc.scalar.copy(v_sb[:, tt, :, 1, HD],
                                           ones_sb[:, 0:NHP])

            # ---- attention (head pairs) + output projection
            with tc.tile_pool(name="attb", bufs=1) as attbp, \
                 tc.tile_pool(name="nrm", bufs=2) as nrmp, \
                 tc.tile_pool(name="oev", bufs=2) as oevp:

                # packed attT: partitions 0:64 = even head, 64:128 = odd head
                attT_sb = attbp.tile([128, NHP, CHUNK], f32r)
                wo_sb = attbp.tile([128, NKD, D], f32r)
                for g in range(NKD):
                    nc.scalar.dma_start(out=wo_sb[:, g, :], in_=wo[:, g, :])

                with tc.tile_pool(name="ps_s", bufs=2, space="PSUM") as ps_sc, \
                     tc.tile_pool(name="ps_at", bufs=3, space="PSUM") as ps_at, \
                     tc.tile_pool(name="ps_bc", bufs=1, space="PSUM") as ps_bc:
                  pending_norm = [None]
                  for hp in range(NHP):
                    att_e = ps_at.tile([128, CHUNK], f32, tag="att")
                    att_o = ps_at.tile([128, CHUNK], f32, tag="att")
                    sc_tiles = {}
                    pt_tiles = {}

                    # groups: merged same-width same-side kb pairs share one
                    # psum tile + one exp; first group covers q[0:512) so the
                    # attV accumulation init touches every column
                    GROUPS = [[4], [5], [6, 7], [0, 1], [2], [3]]

                    def emit_sc(i):
                        kbs = GROUPS[i]
                        qlo, qhi = QRANGE[kbs[0]]
                        wdt = qhi - qlo
                        sc = ps_sc.tile([128, 2, CHUNK], f32, tag="sc")
                        sc_tiles[i] = sc
                        for j, kb in enumerate(kbs):
                            for s in range(2):
                                po = s * 64
                                nc.tensor.matmul(
                                    sc[:, s, j * wdt:(j + 1) * wdt],
                                    kT_sb[po:po + 64, hp,
                                          kb * 128:(kb + 1) * 128],
                                    qT_sb[po:po + 64, hp, qlo:qhi],
                                    start=True, stop=True)
                        # exp for both heads (and both kbs if merged) at once
                        pt = ptp.tile([128, 2, CHUNK], f32r, tag="pt")
                        pt_tiles[i] = pt
                        ew = len(kbs) * wdt
                        nc.scalar.activation(
                            pt[:, :, 0:ew], sc[:, :, 0:ew],
                            mybir.ActivationFunctionType.Exp,
                            bias=kbias_sb[:, kbs[0]:kbs[0] + 1], scale=SCALE)
                        # band-edge masks: one region per kb, both head slots
                        for j, kb in enumerate(kbs):
                            off, m0, m1 = MASKR[kb]
                            off += j * wdt
                            mw = (m1 - m0) * 128
                            msrc = mask_sb[:, m0:m1, :]
                            mbc = bass.AP(tensor=msrc.tensor,
                                          offset=msrc.offset,
                                          ap=[list(msrc.ap[0]), [0, 2]]
                                          + [list(a) for a in msrc.ap[1:]])
                            pslice = pt[:, :, off:off + mw]
                            pv = bass.AP(tensor=pslice.tensor,
                                         offset=pslice.offset,
                                         ap=[list(pslice.ap[0]),
                                             list(pslice.ap[1]),
                                             [128, mw // 128], [1, 128]])
                            nc.vector.tensor_mul(pv, pv, mbc)

                    def emit_att(i):
                        kbs = GROUPS[i]
                        qlo, qhi = QRANGE[kbs[0]]
                        wdt = qhi - qlo
                        pt = pt_tiles.pop(i)
                        sc_tiles.pop(i)
                        for j, kb in enumerate(kbs):
                            first = (i == 0 and j == 0)
                            last = (i == len(GROUPS) - 1
                                    and j == len(kbs) - 1)
                            nc.tensor.matmul(
                                att_e[0:65, qlo:qhi],
                                v_sb[:, kb, hp, 0, :],
                                pt[:, 0, j * wdt:(j + 1) * wdt],
                                start=first, stop=last)
                            nc.tensor.matmul(
                                att_o[0:65, qlo:qhi],
                                v_sb[:, kb, hp, 1, :],
                                pt[:, 1, j * wdt:(j + 1) * wdt],
                                start=first, stop=last)

                    # software-pipelined: run scores 2 groups ahead; the
                    # previous pair's normalize is emitted after this pair's
                    # first scores so PE never stalls on the reciprocal.
                    emit_sc(0)
                    emit_sc(1)
                    if pending_norm[0] is not None:
                        pending_norm[0]()
                    for i in range(len(GROUPS)):
                        if i + 2 < len(GROUPS):
                            emit_sc(i + 2)
                        emit_att(i)

                    # normalize: reciprocal of the sums row (issued now, off
                    # PE), then deferred: broadcast to 64 partitions with a PE
                    # rank-1 outer product (ones x recip) and psum-evicting
                    # multiply. The odd head goes through a staging tile +
                    # partition-shifting DMA to attT rows 64:128.
                    recips = []
                    for s, att_ps in ((0, att_e), (1, att_o)):
                        recip = nrmp.tile([128, CHUNK], f32r, tag="recip")
                        with nc.allow_low_precision(
                                reason="f32r recip row is bit-identical f32"):
                            nc.vector.reciprocal(recip[64:65, :],
                                                 att_ps[64:65, :])
                        recips.append(recip)

                    def norm_closure(hp=hp, att_e=att_e, att_o=att_o,
                                     recips=recips):
                        for s, att_ps, recip in ((0, att_e, recips[0]),
                                                 (1, att_o, recips[1])):
                            bc = ps_bc.tile([128, CHUNK], f32, tag="bc")
                            nc.tensor.matmul(bc[0:64, :], ones64[64:65, :],
                                             recip[64:65, :],
                                             start=True, stop=True)
                            if s == 0:
                                nc.vector.tensor_mul(
                                    attT_sb[0:64, hp, :],
                                    att_ps[0:64, :],
                                    bc[0:64, :])
                            else:
                                stage = nrmp.tile([64, CHUNK], f32r,
                                                  tag="stage")
                                nc.vector.tensor_mul(
                                    stage[:, :], att_ps[0:64, :], bc[0:64, :])
                                nc.sync.dma_start(
                                    out=attT_sb[64:128, hp, :],
                                    in_=stage[:, :])

                    pending_norm[0] = norm_closure
                  pending_norm[0]()

                # ---- output projection: K=128 per head pair
                with tc.tile_pool(name="ps_o", bufs=2, space="PSUM") as ps_o:
                  for eo in range(NKD):
                    ps = ps_o.tile([128, CHUNK], f32, tag="ps")
                    for hp in range(NHP):
                        nc.tensor.matmul(
                            ps[:],
                            wo_sb[:, hp, eo * 128:(eo + 1) * 128],
                            attT_sb[:, hp, :],
                            start=(hp == 0), stop=(hp == NHP - 1))
                    ot = oevp.tile([128, CHUNK], f32, tag="ot")
                    nc.scalar.copy(ot[:], ps[:])
                    nc.scalar.dma_start(out=outT[eo * 128:(eo + 1) * 128, :],
                                        in_=ot[:])

    nc.compile()
    return nc


def _host_inputs(x, w_qkv, w_out):
    x = np.ascontiguousarray(np.asarray(x, dtype=np.float32))
    w_qkv = np.ascontiguousarray(np.asarray(w_qkv, dtype=np.float32))
    w_out = np.ascontiguousarray(np.asarray(w_out, dtype=np.float32))

    wq = w_qkv[:, 0:D]
    wk = w_qkv[:, D:2 * D]
    wv = np.ascontiguousarray(w_qkv[:, 2 * D:3 * D])

    # [co, kd, p, c] layout for per-co-tile streaming loads
    def co_kd(w):
        return np.ascontiguousarray(
            w.reshape(NKD, 128, NKD, 128).transpose(2, 0, 1, 3))

    wq_t, wk_t = co_kd(wq), co_kd(wk)

    # wo pair-major: partitions 0:64 = rows of head 2hp, 64:128 = head 2hp+1
    wo_t = np.ascontiguousarray(
        w_out.reshape(NHP, 2, HD, D).transpose(1, 2, 0, 3).reshape(128, NHP, D))

    r = np.arange(128)[:, None]
    c = np.arange(128)[None, :]
    mask = np.zeros((128, 3, 128), dtype=np.float32)
    mask[:, 0, :] = (r > c).astype(np.float32)
    mask[:, 2, :] = (r <= c).astype(np.float32)
    vones = np.ones((128, NHP), dtype=np.float32)
    vones64 = np.ones((128, HD), dtype=np.float32)

    in_maps = []
    for core in range(NCORES):
        b, qc = divmod(core, 4)
        q0 = qc * CHUNK
        xa = np.zeros((TOK, D), dtype=np.float32)
        lo = max(0, q0 - CHUNK)
        xa[CHUNK - (q0 - lo):] = x[b, lo:q0 + CHUNK]
        kb_bias = np.zeros((128, NKD), dtype=np.float32)
        if qc == 0:
            kb_bias[:, 0:4] = -250.0
        in_maps.append({
            "xT": np.ascontiguousarray(xa.T),
            "wq": wq_t, "wk": wk_t, "wv": wv, "wo": wo_t,
            "mask": mask, "kbias": kb_bias, "vones": vones,
            "vones64": vones64,
        })
    return in_maps


def kernel(x, w_qkv, w_out):
    global _BUILT
    if _BUILT is None:
        _BUILT = _build()
    from concourse.bass_utils import run_bass_kernel_spmd

    in_maps = _host_inputs(x, w_qkv, w_out)
    res = run_bass_kernel_spmd(_BUILT, in_maps, core_ids=list(range(NCORES)))
    out = np.empty((B, T, D), dtype=np.float32)
    for core in range(NCORES):
        b, qc = divmod(core, 4)
        out[b, qc * CHUNK:(qc + 1) * CHUNK, :] = res.results[core]["outT"].T
    return out
